# revision 26
# baseline (speedup 1.0000x reference)
"""CosineAttention Trainium2 kernel (v3 pipeline).

reference:
    xn  = x / max(||x_i||, eps)        # row-normalize
    sim = xn @ xn.T                    # [N, N]
    out = sigmoid(sim @ x)             # [N, D]

Matmul associativity: sim @ x = xn @ A with A = xn^T x  [D, D]
(O(N D^2) instead of O(N^2 D)).  A is symmetric, so only the upper
block-triangle of A is computed and AllReduced (44% less PE work and
44% less collective wire); the lower blocks are reconstructed locally
by PE transposes after the reduction.

Distribution (8 cores, 1-D row-parallel): core c owns 1024 rows.
    A_c = xn_c^T x_c   (partial, upper triangle, strip by strip)
    A   = AllReduce(A_c)  -- 3 grouped fp16 collectives, pipelined
    outT_c = sigmoid(A^T xn_c^T) = sigmoid(xn_c A)^T
The output is produced transposed ([D, rows]) so MM2 can chase the
AllReduce ladder with full-width (512) moving matmuls; the host
transposes back when assembling the full result.

Schedule highlights:
  - x arrives fp16 (host cast): 2 MB load over SP + ACT + gpsimd DGE.
  - Strips 0-2 of A accumulate in 6 PSUM banks *during* the load
    (matmuls per arriving row-chunk); xn^T transposes also run in the
    load shadow (2 PSUM banks).  Strips 3-7 run right after.
  - Strip groups {0-2}, {3-5}, {6-7} each drain into one contiguous
    fp16 staging tile and ship with a single 128-packet DMA, then
    AllReduce as a group; collectives pipeline on the CC core.
  - Output slice dc needs strips 0..dc only, so MM2 slices chase the
    group ladder; outT stores are coalesced two slices per DMA.
"""

import sys

if "/opt/trn_rl_repo" not in sys.path:
    sys.path.insert(0, "/opt/trn_rl_repo")

import numpy as np

N, D = 8192, 1024
NCORES = 8
ROWS = N // NCORES  # 1024 rows per core
P = 128
MC = ROWS // P  # 8 row chunks per core
KC = D // P  # 8 k strips (rows of A)
SHADOW_STRIPS = 3  # strips accumulated during the x load (6 PSUM banks)
GROUPS = [(0, 1, 2), (3, 4, 5), (6, 7)]  # AllReduce strip groups

_prog_cache = {}


def _strip_w(kc):
    return D - P * kc


def _strip_chunks(kc):
    """(col_start, width) pieces of strip kc, each <= 512 (one PSUM bank)."""
    w = _strip_w(kc)
    c0 = P * kc
    out = [(c0, min(512, w))]
    if w > 512:
        out.append((c0 + 512, w - 512))
    return out


def _group_of(kc):
    for g, strips in enumerate(GROUPS):
        if kc in strips:
            return g
    raise ValueError(kc)


def _group_w(g):
    return sum(_strip_w(kc) for kc in GROUPS[g])


def _strip_off(kc):
    """Offset of strip kc inside its group staging tile."""
    g = _group_of(kc)
    off = 0
    for s in GROUPS[g]:
        if s == kc:
            return off
        off += _strip_w(s)
    raise ValueError(kc)


def _build_program():
    import concourse.bacc as bacc
    import concourse.mybir as mybir
    import concourse.tile as tile
    from concourse.masks import make_identity

    f32 = mybir.dt.float32
    f16 = mybir.dt.float16
    AF = mybir.ActivationFunctionType
    GRP = [list(range(NCORES))]

    nc = bacc.Bacc(
        trn_type="TRN2", target_bir_lowering=False, debug=False, num_devices=NCORES
    )
    xloc_d = nc.dram_tensor("xloc", [ROWS, D], f16, kind="ExternalInput").ap()
    outT_d = nc.dram_tensor("outT", [D, ROWS], f16, kind="ExternalOutput").ap()

    with tile.TileContext(nc) as tc:
        with (
            tc.tile_pool(name="singles", bufs=1) as singles,
            tc.tile_pool(name="sq", bufs=2) as sq_pool,
            tc.tile_pool(name="ast", bufs=2) as ast_pool,
            tc.tile_pool(name="mir", bufs=2) as mir_pool,
            tc.tile_pool(name="ot", bufs=3) as ot_pool,
            tc.tile_pool(name="small", bufs=4) as small,
            tc.tile_pool(name="dram", bufs=1, space="DRAM") as dram,
            tc.tile_pool(name="ps_s", bufs=6, space="PSUM") as ps_s,
            tc.tile_pool(name="ps_t", bufs=2, space="PSUM") as ps_t,
        ):
            ident_g = singles.tile([P, P], f32)
            make_identity(nc, ident_g)
            ident16 = singles.tile([P, P], f16)
            nc.vector.tensor_copy(out=ident16, in_=ident_g)

            xs = singles.tile([P, MC, D], f16)  # x row chunks (fp16)
            xn = singles.tile([P, MC, D], f16)  # x / ||row||
            BT = singles.tile([P, KC, ROWS], f16)  # xn^T (k on partitions)
            inv = singles.tile([P, MC], f32)  # 1 / ||row||
            a_sb = [
                singles.tile([P, _group_w(g)], f16, name=f"a_sb{g}")
                for g in range(len(GROUPS))
            ]

            bar_in = dram.tile([P, 4], f32, tag="bar_in", name="bar_in")
            bar_out = dram.tile(
                [P, 4 * NCORES],
                f32,
                tag="bar_out",
                name="bar_out",
                addr_space="Shared",
            )
            a_part = [
                dram.tile([P, _group_w(g)], f16, tag=f"apart{g}", name=f"a_part{g}")
                for g in range(len(GROUPS))
            ]
            a_red = [
                dram.tile(
                    [P, _group_w(g)],
                    f16,
                    tag=f"ared{g}",
                    name=f"a_red{g}",
                    addr_space="Shared",
                )
                for g in range(len(GROUPS))
            ]

            # ---- t=0 collective: warms the CC stream (~40us bootstrap runs
            # from the first trigger), so the real AllReduces start early.
            # AllGather only copies, so the uninitialized input is harmless
            # and skipping its init puts the trigger at the earliest slot ----
            nc.gpsimd.collective_compute(
                "AllGather",
                mybir.AluOpType.bypass,
                replica_groups=GRP,
                ins=[bar_in[:].opt()],
                outs=[bar_out[:].opt()],
            )

            # shadow-strip PSUM tiles (live across the whole load)
            shadow_ps = {}
            for kc in range(SHADOW_STRIPS):
                for c0, w in _strip_chunks(kc):
                    shadow_ps[(kc, c0)] = ps_s.tile(
                        [P, w], f32, tag="acc", name=f"acc{kc}_{c0}"
                    )

            # ---- load + normalize + shadow MM-A + xn^T transposes ----
            for rc in range(MC):
                if rc in (2, 5):
                    nc.gpsimd.dma_start(
                        out=xs[:, rc, :], in_=xloc_d[rc * P : (rc + 1) * P, :]
                    )
                else:
                    deng = nc.sync if rc % 2 == 0 else nc.scalar
                    deng.dma_start(
                        out=xs[:, rc, :], in_=xloc_d[rc * P : (rc + 1) * P, :]
                    )
                sq = sq_pool.tile([P, D], f16, tag="sq")
                ssq = small.tile([P, 1], f32, tag="ssq")
                nc.scalar.activation(
                    out=sq, in_=xs[:, rc, :], func=AF.Square, accum_out=ssq
                )
                nrm = small.tile([P, 1], f32, tag="nrm")
                nc.scalar.activation(out=nrm, in_=ssq, func=AF.Sqrt)
                nc.vector.reciprocal(inv[:, rc : rc + 1], nrm)
                nc.vector.tensor_scalar_mul(
                    out=xn[:, rc, :], in0=xs[:, rc, :], scalar1=inv[:, rc : rc + 1]
                )
                for kc in range(SHADOW_STRIPS):
                    for c0, w in _strip_chunks(kc):
                        nc.tensor.matmul(
                            shadow_ps[(kc, c0)],
                            xn[:, rc, kc * P : (kc + 1) * P],
                            xs[:, rc, c0 : c0 + w],
                            start=(rc == 0),
                            stop=(rc == MC - 1),
                        )
                # all 8 transposes of this chunk into one fp16 PSUM bank
                pst = ps_t.tile([P, KC * P], f16, tag="pst")
                for kc in range(KC):
                    nc.tensor.transpose(
                        pst[:, kc * P : (kc + 1) * P],
                        xn[:, rc, kc * P : (kc + 1) * P],
                        ident16,
                    )
                nc.vector.tensor_copy(
                    out=BT[:, :, rc * P : (rc + 1) * P],
                    in_=pst.rearrange("p (k q) -> p k q", k=KC),
                )

            # ---- drain strips into group staging tiles; AllReduce per group ----
            ast_tiles = {}

            def drain_strip(kc, tiles):
                g = _group_of(kc)
                if g not in ast_tiles:
                    ast_tiles[g] = ast_pool.tile(
                        [P, _group_w(g)], f16, tag="ast", name=f"ast{g}"
                    )
                ast = ast_tiles[g]
                lo = _strip_off(kc)
                for i, (c0, pt) in enumerate(tiles):
                    cw = pt.shape[-1]
                    dst = ast[:, lo + c0 - P * kc : lo + c0 - P * kc + cw]
                    if i == 0:
                        nc.scalar.activation(out=dst, in_=pt, func=AF.Copy)
                    else:
                        nc.vector.tensor_copy(out=dst, in_=pt)

            def ship_group(g):
                seng = nc.sync if g % 2 == 0 else nc.scalar
                seng.dma_start(out=a_part[g][:], in_=ast_tiles[g])
                nc.gpsimd.collective_compute(
                    "AllReduce",
                    mybir.AluOpType.add,
                    replica_groups=GRP,
                    ins=[a_part[g][:].opt()],
                    outs=[a_red[g][:].opt()],
                )

            for kc in range(SHADOW_STRIPS):
                drain_strip(
                    kc, [(c0, shadow_ps[(kc, c0)]) for c0, _ in _strip_chunks(kc)]
                )
            ship_group(0)

            # ---- strips 3..7 (x fully resident now) ----
            post_ps = {}
            for kc in range(SHADOW_STRIPS, KC):
                for c0, w in _strip_chunks(kc):
                    post_ps[(kc, c0)] = ps_s.tile(
                        [P, w], f32, tag="acc", name=f"acc{kc}_{c0}"
                    )
            for rc in range(MC):
                for (kc, c0), pt in post_ps.items():
                    w = pt.shape[-1]
                    nc.tensor.matmul(
                        pt,
                        xn[:, rc, kc * P : (kc + 1) * P],
                        xs[:, rc, c0 : c0 + w],
                        start=(rc == 0),
                        stop=(rc == MC - 1),
                    )
            for kc in range(SHADOW_STRIPS, KC):
                drain_strip(kc, [(c0, post_ps[(kc, c0)]) for c0, _ in _strip_chunks(kc)])
            for g in range(1, len(GROUPS)):
                ship_group(g)

            # ---- MM2: outT slice dc needs strips 0..dc only ----
            def load_a_sb(g):
                nc.scalar.dma_start(out=a_sb[g], in_=a_red[g][:])

            load_a_sb(0)

            def stat_ap(kc, dc):
                """A[kc-slice, dc-cols] as [128, 128] SBUF slice (kc <= dc)."""
                g = _group_of(kc)
                lo = _strip_off(kc) + (dc - kc) * P
                return a_sb[g][:, lo : lo + P]

            def slice_mirrors(dc, kc_lo):
                """Transpose blocks (dc, kc) for kc in [kc_lo, KC) into a tile."""
                nmir = KC - kc_lo
                mir = mir_pool.tile(
                    [P, nmir * P], f16, tag="mir", name=f"mir{dc}_{kc_lo}"
                )
                pst = ps_t.tile([P, KC * P], f16, tag="pst")
                for j in range(nmir):
                    kc = kc_lo + j
                    nc.tensor.transpose(
                        pst[:, j * P : (j + 1) * P], stat_ap(dc, kc), ident16
                    )
                nc.vector.tensor_copy(out=mir, in_=pst[:, : nmir * P])
                return mir

            def slice_mm(dc, ps2, kc_lo, kc_hi, mir=None, mir_lo=0):
                for kc in range(kc_lo, kc_hi + 1):
                    if kc <= dc:
                        stat = stat_ap(kc, dc)
                    else:
                        stat = mir[:, (kc - mir_lo) * P : (kc - mir_lo + 1) * P]
                    for h in range(2):
                        nc.tensor.matmul(
                            ps2[h],
                            stat,
                            BT[:, kc, h * 512 : (h + 1) * 512],
                            start=(kc == 0),
                            stop=(kc == KC - 1),
                        )

            def slice_finish(dc, ps2, split_store=False):
                ot = ot_pool.tile([P, ROWS], f16, tag="ot", name=f"ot{dc}")
                for h in range(2):
                    nc.scalar.activation(
                        out=ot[:, h * 512 : (h + 1) * 512],
                        in_=ps2[h],
                        func=AF.Sigmoid,
                    )
                    if split_store:
                        eng = nc.sync if h == 0 else nc.scalar
                        eng.dma_start(
                            out=outT_d[dc * P : (dc + 1) * P, h * 512 : (h + 1) * 512],
                            in_=ot[:, h * 512 : (h + 1) * 512],
                        )
                if not split_store:
                    seng = nc.sync if dc % 2 == 0 else nc.scalar
                    seng.dma_start(out=outT_d[dc * P : (dc + 1) * P, :], in_=ot)

            def new_ps2(dc):
                return [
                    ps_s.tile([P, 512], f32, tag="acc", name=f"ps2_{dc}_{h}")
                    for h in range(2)
                ]

            # slices 0..4: fully gated by AllReduce groups 0/1
            for dc in range(5):
                gd = _group_of(dc)
                if _group_of(dc + 1) != gd:
                    load_a_sb(gd + 1)
                mir = slice_mirrors(dc, dc + 1)
                ps2 = new_ps2(dc)
                slice_mm(dc, ps2, 0, KC - 1, mir=mir, mir_lo=dc + 1)
                slice_finish(dc, ps2)
            # park slices 6/7: their kc<=5 contraction needs only groups 0/1,
            # so it runs while the {s6,s7} AllReduce is still in flight
            ps2_6 = new_ps2(6)
            slice_mm(6, ps2_6, 0, 5)
            ps2_7 = new_ps2(7)
            slice_mm(7, ps2_7, 0, 5)
            # slice 5 (mirrors come from strip 5, group 1); group 2's a_sb
            # rides the idle software DGE so sigmoids don't queue behind it
            nc.gpsimd.dma_start(out=a_sb[2], in_=a_red[2][:])
            mir5 = slice_mirrors(5, 6)
            ps2_5 = new_ps2(5)
            slice_mm(5, ps2_5, 0, KC - 1, mir=mir5, mir_lo=6)
            slice_finish(5, ps2_5)
            # slice 6/7 remainders: only kc=6,7 plus slice 6's single mirror
            mir6 = slice_mirrors(6, 7)
            slice_mm(6, ps2_6, 6, KC - 1, mir=mir6, mir_lo=7)
            slice_finish(6, ps2_6)
            slice_mm(7, ps2_7, 6, KC - 1)
            slice_finish(7, ps2_7, split_store=True)

    nc.compile()
    return nc


def get_program():
    if "nc" not in _prog_cache:
        _prog_cache["nc"] = _build_program()
    return _prog_cache["nc"]


def _warm_devices():
    """Warm per-device dispatch paths to reduce core-launch stagger."""
    try:
        import jax

        devs = jax.devices()[:NCORES]
        if len(devs) < NCORES:
            return
        out = jax.pmap(lambda x: x + 1.0)(np.zeros((len(devs), 8), np.float32))
        out.block_until_ready()
    except Exception:
        pass


def kernel(x: np.ndarray, W: np.ndarray, _collect=None) -> np.ndarray:
    """Full-input / full-output entry point. W is an unused declared param."""
    from concourse.bass_utils import run_bass_kernel_spmd

    nc = get_program()
    _warm_devices()
    xh = np.ascontiguousarray(np.asarray(x, dtype=np.float32)).astype(np.float16)
    in_maps = [{"xloc": xh[c * ROWS : (c + 1) * ROWS]} for c in range(NCORES)]
    res = run_bass_kernel_spmd(
        nc, in_maps, list(range(NCORES)), trace=bool(_collect is not None)
    )
    if _collect is not None:
        _collect["results"] = res
    return np.concatenate(
        [res.results[c]["outT"].T.astype(np.float32) for c in range(NCORES)], axis=0
    )


if __name__ == "__main__":
    get_program()
    print("program built OK")


# revision 29
# speedup vs baseline: 1.0433x; 1.0433x over previous
"""CosineAttention Trainium2 kernel (v3 pipeline).

reference:
    xn  = x / max(||x_i||, eps)        # row-normalize
    sim = xn @ xn.T                    # [N, N]
    out = sigmoid(sim @ x)             # [N, D]

Matmul associativity: sim @ x = xn @ A with A = xn^T x  [D, D]
(O(N D^2) instead of O(N^2 D)).  A is symmetric, so only the upper
block-triangle of A is computed and AllReduced (44% less PE work and
44% less collective wire); the lower blocks are reconstructed locally
by PE transposes after the reduction.

Distribution (8 cores, 1-D row-parallel): core c owns 1024 rows.
    A_c = xn_c^T x_c   (partial, upper triangle, strip by strip)
    A   = AllReduce(A_c)  -- 3 grouped fp16 collectives, pipelined
    outT_c = sigmoid(A^T xn_c^T) = sigmoid(xn_c A)^T
The output is produced transposed ([D, rows]) so MM2 can chase the
AllReduce ladder with full-width (512) moving matmuls; the host
transposes back when assembling the full result.

Schedule highlights:
  - x arrives fp16 (host cast): 2 MB load over SP + ACT + gpsimd DGE.
  - Strips 0-2 of A accumulate in 6 PSUM banks *during* the load
    (matmuls per arriving row-chunk); xn^T transposes also run in the
    load shadow (2 PSUM banks).  Strips 3-7 run right after.
  - Strip groups {0-2}, {3-5}, {6-7} each drain into one contiguous
    fp16 staging tile and ship with a single 128-packet DMA, then
    AllReduce as a group; collectives pipeline on the CC core.
  - Output slice dc needs strips 0..dc only, so MM2 slices chase the
    group ladder; outT stores are coalesced two slices per DMA.
"""

import sys

if "/opt/trn_rl_repo" not in sys.path:
    sys.path.insert(0, "/opt/trn_rl_repo")

import numpy as np

N, D = 8192, 1024
NCORES = 8
ROWS = N // NCORES  # 1024 rows per core
P = 128
MC = ROWS // P  # 8 row chunks per core
KC = D // P  # 8 k strips (rows of A)
SHADOW_STRIPS = 3  # strips accumulated during the x load (6 PSUM banks)
GROUPS = [(0, 1, 2), (3, 4, 5), (6, 7)]  # AllReduce strip groups

_prog_cache = {}


def _strip_w(kc):
    return D - P * kc


def _strip_chunks(kc):
    """(col_start, width) pieces of strip kc, each <= 512 (one PSUM bank)."""
    w = _strip_w(kc)
    c0 = P * kc
    out = [(c0, min(512, w))]
    if w > 512:
        out.append((c0 + 512, w - 512))
    return out


def _group_of(kc):
    for g, strips in enumerate(GROUPS):
        if kc in strips:
            return g
    raise ValueError(kc)


def _group_w(g):
    return sum(_strip_w(kc) for kc in GROUPS[g])


def _strip_off(kc):
    """Offset of strip kc inside its group staging tile."""
    g = _group_of(kc)
    off = 0
    for s in GROUPS[g]:
        if s == kc:
            return off
        off += _strip_w(s)
    raise ValueError(kc)


def _build_program():
    import concourse.bacc as bacc
    import concourse.mybir as mybir
    import concourse.tile as tile
    from concourse.masks import make_identity

    f32 = mybir.dt.float32
    f16 = mybir.dt.float16
    AF = mybir.ActivationFunctionType
    GRP = [list(range(NCORES))]

    nc = bacc.Bacc(
        trn_type="TRN2", target_bir_lowering=False, debug=False, num_devices=NCORES
    )
    xloc_d = nc.dram_tensor("xloc", [ROWS, D], f16, kind="ExternalInput").ap()
    outT_d = nc.dram_tensor("outT", [D, ROWS], f16, kind="ExternalOutput").ap()

    with tile.TileContext(nc) as tc:
        with (
            tc.tile_pool(name="singles", bufs=1) as singles,
            tc.tile_pool(name="sq", bufs=2) as sq_pool,
            tc.tile_pool(name="ast", bufs=2) as ast_pool,
            tc.tile_pool(name="mir", bufs=2) as mir_pool,
            tc.tile_pool(name="ot", bufs=3) as ot_pool,
            tc.tile_pool(name="small", bufs=4) as small,
            tc.tile_pool(name="dram", bufs=1, space="DRAM") as dram,
            tc.tile_pool(name="ps_s", bufs=6, space="PSUM") as ps_s,
            tc.tile_pool(name="ps_t", bufs=2, space="PSUM") as ps_t,
        ):
            ident_g = singles.tile([P, P], f32)
            make_identity(nc, ident_g)
            ident16 = singles.tile([P, P], f16)
            nc.vector.tensor_copy(out=ident16, in_=ident_g)

            xs = singles.tile([P, MC, D], f16)  # x row chunks (fp16)
            xn = singles.tile([P, MC, D], f16)  # x / ||row||
            BT = singles.tile([P, KC, ROWS], f16)  # xn^T (k on partitions)
            inv = singles.tile([P, MC], f32)  # 1 / ||row||
            a_sb = [
                singles.tile([P, _group_w(g)], f16, name=f"a_sb{g}")
                for g in range(len(GROUPS))
            ]

            bar_in = dram.tile([P, 4], f32, tag="bar_in", name="bar_in")
            bar_out = dram.tile(
                [P, 4 * NCORES],
                f32,
                tag="bar_out",
                name="bar_out",
                addr_space="Shared",
            )
            a_part = [
                dram.tile([P, _group_w(g)], f16, tag=f"apart{g}", name=f"a_part{g}")
                for g in range(len(GROUPS))
            ]
            a_red = [
                dram.tile(
                    [P, _group_w(g)],
                    f16,
                    tag=f"ared{g}",
                    name=f"a_red{g}",
                    addr_space="Shared",
                )
                for g in range(len(GROUPS))
            ]

            # ---- t=0 collective: warms the CC stream (~40us bootstrap runs
            # from the first trigger), so the real AllReduces start early.
            # AllGather only copies, so the uninitialized input is harmless
            # and skipping its init puts the trigger at the earliest slot ----
            nc.gpsimd.collective_compute(
                "AllGather",
                mybir.AluOpType.bypass,
                replica_groups=GRP,
                ins=[bar_in[:].opt()],
                outs=[bar_out[:].opt()],
            )

            # shadow-strip PSUM tiles (live across the whole load)
            shadow_ps = {}
            for kc in range(SHADOW_STRIPS):
                for c0, w in _strip_chunks(kc):
                    shadow_ps[(kc, c0)] = ps_s.tile(
                        [P, w], f32, tag="acc", name=f"acc{kc}_{c0}"
                    )

            # ---- load + normalize + shadow MM-A + xn^T transposes ----
            for rc in range(MC):
                if rc in (2, 5):
                    nc.gpsimd.dma_start(
                        out=xs[:, rc, :], in_=xloc_d[rc * P : (rc + 1) * P, :]
                    )
                else:
                    deng = nc.sync if rc % 2 == 0 else nc.scalar
                    deng.dma_start(
                        out=xs[:, rc, :], in_=xloc_d[rc * P : (rc + 1) * P, :]
                    )
                sq = sq_pool.tile([P, D], f16, tag="sq")
                ssq = small.tile([P, 1], f32, tag="ssq")
                nc.scalar.activation(
                    out=sq, in_=xs[:, rc, :], func=AF.Square, accum_out=ssq
                )
                nrm = small.tile([P, 1], f32, tag="nrm")
                nc.scalar.activation(out=nrm, in_=ssq, func=AF.Sqrt)
                nc.vector.reciprocal(inv[:, rc : rc + 1], nrm)
                nc.vector.tensor_scalar_mul(
                    out=xn[:, rc, :], in0=xs[:, rc, :], scalar1=inv[:, rc : rc + 1]
                )
                for kc in range(SHADOW_STRIPS):
                    for c0, w in _strip_chunks(kc):
                        nc.tensor.matmul(
                            shadow_ps[(kc, c0)],
                            xn[:, rc, kc * P : (kc + 1) * P],
                            xs[:, rc, c0 : c0 + w],
                            start=(rc == 0),
                            stop=(rc == MC - 1),
                        )
                # all 8 transposes of this chunk into one fp16 PSUM bank
                pst = ps_t.tile([P, KC * P], f16, tag="pst")
                for kc in range(KC):
                    nc.tensor.transpose(
                        pst[:, kc * P : (kc + 1) * P],
                        xn[:, rc, kc * P : (kc + 1) * P],
                        ident16,
                    )
                nc.vector.tensor_copy(
                    out=BT[:, :, rc * P : (rc + 1) * P],
                    in_=pst.rearrange("p (k q) -> p k q", k=KC),
                )

            # ---- drain strips into group staging tiles; AllReduce per group ----
            ast_tiles = {}

            def drain_strip(kc, tiles):
                g = _group_of(kc)
                if g not in ast_tiles:
                    ast_tiles[g] = ast_pool.tile(
                        [P, _group_w(g)], f16, tag="ast", name=f"ast{g}"
                    )
                ast = ast_tiles[g]
                lo = _strip_off(kc)
                for i, (c0, pt) in enumerate(tiles):
                    cw = pt.shape[-1]
                    dst = ast[:, lo + c0 - P * kc : lo + c0 - P * kc + cw]
                    if i == 0:
                        nc.scalar.activation(out=dst, in_=pt, func=AF.Copy)
                    else:
                        nc.vector.tensor_copy(out=dst, in_=pt)

            def ship_group(g):
                seng = nc.sync if g % 2 == 0 else nc.scalar
                seng.dma_start(out=a_part[g][:], in_=ast_tiles[g])
                nc.gpsimd.collective_compute(
                    "AllReduce",
                    mybir.AluOpType.add,
                    replica_groups=GRP,
                    ins=[a_part[g][:].opt()],
                    outs=[a_red[g][:].opt()],
                )

            for kc in range(SHADOW_STRIPS):
                drain_strip(
                    kc, [(c0, shadow_ps[(kc, c0)]) for c0, _ in _strip_chunks(kc)]
                )
            ship_group(0)

            # ---- strips 3..7 (x fully resident now) ----
            post_ps = {}
            for kc in range(SHADOW_STRIPS, KC):
                for c0, w in _strip_chunks(kc):
                    post_ps[(kc, c0)] = ps_s.tile(
                        [P, w], f32, tag="acc", name=f"acc{kc}_{c0}"
                    )
            for rc in range(MC):
                for (kc, c0), pt in post_ps.items():
                    w = pt.shape[-1]
                    nc.tensor.matmul(
                        pt,
                        xn[:, rc, kc * P : (kc + 1) * P],
                        xs[:, rc, c0 : c0 + w],
                        start=(rc == 0),
                        stop=(rc == MC - 1),
                    )
            for kc in range(SHADOW_STRIPS, KC):
                drain_strip(kc, [(c0, post_ps[(kc, c0)]) for c0, _ in _strip_chunks(kc)])
            for g in range(1, len(GROUPS)):
                ship_group(g)

            # ---- MM2: outT slice dc needs strips 0..dc only ----
            def load_a_sb(g):
                nc.scalar.dma_start(out=a_sb[g], in_=a_red[g][:])

            load_a_sb(0)

            def stat_ap(kc, dc):
                """A[kc-slice, dc-cols] as [128, 128] SBUF slice (kc <= dc)."""
                g = _group_of(kc)
                lo = _strip_off(kc) + (dc - kc) * P
                return a_sb[g][:, lo : lo + P]

            def slice_mirrors(dc, kc_lo):
                """Transpose blocks (dc, kc) for kc in [kc_lo, KC) into a tile."""
                nmir = KC - kc_lo
                mir = mir_pool.tile(
                    [P, nmir * P], f16, tag="mir", name=f"mir{dc}_{kc_lo}"
                )
                pst = ps_t.tile([P, KC * P], f16, tag="pst")
                for j in range(nmir):
                    kc = kc_lo + j
                    nc.tensor.transpose(
                        pst[:, j * P : (j + 1) * P], stat_ap(dc, kc), ident16
                    )
                nc.vector.tensor_copy(out=mir, in_=pst[:, : nmir * P])
                return mir

            def slice_mm(dc, ps2, kc_lo, kc_hi, mir=None, mir_lo=0):
                for kc in range(kc_lo, kc_hi + 1):
                    if kc <= dc:
                        stat = stat_ap(kc, dc)
                    else:
                        stat = mir[:, (kc - mir_lo) * P : (kc - mir_lo + 1) * P]
                    for h in range(2):
                        nc.tensor.matmul(
                            ps2[h],
                            stat,
                            BT[:, kc, h * 512 : (h + 1) * 512],
                            start=(kc == 0),
                            stop=(kc == KC - 1),
                        )

            def slice_finish(dc, ps2, split_store=False):
                ot = ot_pool.tile([P, ROWS], f16, tag="ot", name=f"ot{dc}")
                for h in range(2):
                    nc.scalar.activation(
                        out=ot[:, h * 512 : (h + 1) * 512],
                        in_=ps2[h],
                        func=AF.Sigmoid,
                    )
                    if split_store:
                        eng = nc.sync if h == 0 else nc.scalar
                        eng.dma_start(
                            out=outT_d[dc * P : (dc + 1) * P, h * 512 : (h + 1) * 512],
                            in_=ot[:, h * 512 : (h + 1) * 512],
                        )
                if not split_store:
                    seng = nc.sync if dc % 2 == 0 else nc.scalar
                    seng.dma_start(out=outT_d[dc * P : (dc + 1) * P, :], in_=ot)

            def new_ps2(dc):
                return [
                    ps_s.tile([P, 512], f32, tag="acc", name=f"ps2_{dc}_{h}")
                    for h in range(2)
                ]

            # slices in AllReduce-ladder order: slice dc needs strips 0..dc
            for dc in range(KC):
                gd = _group_of(dc)
                if dc + 1 < KC and _group_of(dc + 1) != gd:
                    load_a_sb(gd + 1)
                mir = slice_mirrors(dc, dc + 1) if dc + 1 < KC else None
                ps2 = new_ps2(dc)
                slice_mm(dc, ps2, 0, KC - 1, mir=mir, mir_lo=dc + 1)
                slice_finish(dc, ps2, split_store=(dc == KC - 1))

    nc.compile()
    return nc


def get_program():
    if "nc" not in _prog_cache:
        _prog_cache["nc"] = _build_program()
    return _prog_cache["nc"]


def _warm_devices():
    """Warm per-device dispatch paths to reduce core-launch stagger."""
    try:
        import jax

        devs = jax.devices()[:NCORES]
        if len(devs) < NCORES:
            return
        out = jax.pmap(lambda x: x + 1.0)(np.zeros((len(devs), 8), np.float32))
        out.block_until_ready()
    except Exception:
        pass


def kernel(x: np.ndarray, W: np.ndarray, _collect=None) -> np.ndarray:
    """Full-input / full-output entry point. W is an unused declared param."""
    from concourse.bass_utils import run_bass_kernel_spmd

    nc = get_program()
    _warm_devices()
    xh = np.ascontiguousarray(np.asarray(x, dtype=np.float32)).astype(np.float16)
    in_maps = [{"xloc": xh[c * ROWS : (c + 1) * ROWS]} for c in range(NCORES)]
    res = run_bass_kernel_spmd(
        nc, in_maps, list(range(NCORES)), trace=bool(_collect is not None)
    )
    if _collect is not None:
        _collect["results"] = res
    return np.concatenate(
        [res.results[c]["outT"].T.astype(np.float32) for c in range(NCORES)], axis=0
    )


if __name__ == "__main__":
    get_program()
    print("program built OK")
